# revision 37
# baseline (speedup 1.0000x reference)
"""Trainium2 Bass kernel for nn_AIGGenerator (GCN encode -> score matrix ->
prefix-masked top-2 -> inversion-bit MLP), SPMD across 8 NeuronCores.

Sharding: feature pipeline (GCN convs, node_proj) is node-sharded contiguously
(core c owns sorted-node rows [1024c, 1024c+1024)); the [N,N] score/top-k phase
is row-sharded with depth-interleaved 128-row blocks (core c scores blocks
{8j+c}) for load balance, exploiting that node_depth is sorted so the candidate
mask is a prefix per row and deep column tiles can be skipped.

v3 restructure vs v2 (907us -> target ~400us):
  - cmat host-pretiled [128, 64, 1024] so every DMA line is 2-8KB contiguous
    (v2's strided loads were descriptor-bound at ~96ns/desc: conv1 took 90us
    for 48us of matmul and the first matmul waited 56us for chunk 0).
  - degree normalization (rsqrt/recip) computed on host in f32; kills the
    55us Vector-bound startup (single-partition reciprocals + Newton).
  - AG1 payload packed partition-major [128, 2048] (one contiguous DMA in,
    per-shard contiguous reads out).
  - AG2 payload = [h node-major | Hs^T] instead of [h | h^T]: Hs^T is computed
    per-shard BEFORE the collective (2 matmuls) so the 16 post-collective f32
    matmuls of v2's phase 4 become 8 plain DMA loads; h^T-derived per-block
    tgt/q projections come from PE-transposing the h section.
  - phase 5: blocks processed in descending column-count order (expensive
    FIND_INDEX8 scans overlap earlier blocks; the serial tail is the cheapest
    block); score rows double-buffered; masked tiles fused to ONE DVE op
    (min(S, (thr_v - depth_u + 0.5)*1e30) via scalar_tensor_tensor); clean
    gather layout (no scramble).

Precision: the top-2 index selection is extremely tie-sensitive (min top1-top2
gap ~8e-6, min top2-top3 gap ~2e-8 at score scale ~0.2), so the h -> scores
chain is kept at f32-exact grade: GCN aggregation uses the exact-bf16
edge-count matrix with 2-way bf16 splits of the activations, everything else
native f32 TensorE matmul, and PSUM accumulation orders are kept identical to
v2 (which matched the reference top-2 exactly).
"""
import os
import numpy as np
import ml_dtypes

import concourse.bass as bass
import concourse.mybir as mybir
import concourse.tile as tile
from concourse import bacc
from concourse.bass import ds
import concourse.bass_utils as bass_utils
from concourse.masks import make_identity

F32 = mybir.dt.float32
BF16 = mybir.dt.bfloat16
I32 = mybir.dt.int32
I16 = mybir.dt.int16
U32 = mybir.dt.uint32
OP = mybir.AluOpType
AF = mybir.ActivationFunctionType

N = 8192
H = 128
Z = 128
NCORES = 8
P = 128
NT = N // P            # 64 node tiles
VS = N // NCORES       # 1024 nodes per core shard
JB = 8                 # score blocks per core
TS = 512               # score column tile
CHT = 2                # cmat tiles per streamed chunk
NEG = -1e30
BIGM = 1e30            # mask scale for the fused min-mask
DEPTH_PERTURB = 2
NSPL = 2               # bf16 splits of conv2 input activations

LAST_RESULT = None     # BassKernelResults of the most recent run (for test.py)


def _split3(nc, dst3, src_f32, tmp_a, tmp_b):
    """bf16 triple-split: sum of parts == src to ~2^-24 relative."""
    s0, s1, s2 = dst3
    nc.vector.tensor_copy(s0, src_f32)              # hi (bf16 rne)
    nc.vector.tensor_copy(tmp_a, s0)                # hi back to f32
    nc.vector.tensor_tensor(tmp_a, src_f32, tmp_a, OP.subtract)   # e1
    nc.vector.tensor_copy(s1, tmp_a)                # mid
    nc.vector.tensor_copy(tmp_b, s1)
    nc.vector.tensor_tensor(tmp_b, tmp_a, tmp_b, OP.subtract)     # e2
    nc.vector.tensor_copy(s2, tmp_b)                # lo


def _split2(nc, dst2, src_f32, tmp_a):
    """bf16 double-split: sum of parts == src to ~2^-17 relative."""
    s0, s1 = dst2
    nc.vector.tensor_copy(s0, src_f32)              # hi (bf16 rne)
    nc.vector.tensor_copy(tmp_a, s0)                # hi back to f32
    nc.vector.tensor_tensor(tmp_a, src_f32, tmp_a, OP.subtract)   # residual
    nc.vector.tensor_copy(s1, tmp_a)                # lo


def build_program(T, TM, TORD, MS, inv2_b_val):
    """T[j]: number of 512-wide score column tiles for block-group j.
    TM[j]: first tile index that needs depth masking.
    TORD: block processing order (descending T).
    MS: max masked-tile span (T[j]-TM[j]) across blocks."""
    nc = bacc.Bacc("TRN2", target_bir_lowering=False, debug=False,
                   num_devices=NCORES)

    def inp(name, shape, dt):
        return nc.dram_tensor(name, shape, dt, kind="ExternalInput")

    cmat_t = inp("cmat_t", [P, NT, VS], BF16)     # C^T columns, node-tiled
    x_ownT_div = inp("x_ownT_div", [P, VS], F32)  # rows 0:2 = (x/deg)own^T
    p_all = inp("p_all", [P, NT, 2], F32)         # rsq*x all nodes, tiled
    rsq_own = inp("rsq_own", [1, VS], F32)
    inv_own = inp("inv_own", [1, VS], F32)
    rsq_ot = inp("rsq_ot", [P, JB], F32)          # own shard tiled
    depneg_rep = inp("depneg_rep", [P, N], BF16)  # (0.5-depth)*1e30 bcast
    dotb = inp("dotb", [P, JB], F32)              # (depth_rows+1)*1e30
    dot_t = inp("dot_t", [P, JB], F32)            # block-row depth (valid)
    typ_t = inp("typ_t", [P, JB], F32)            # block-row type (valid)
    w1t6 = inp("w1t6", [P, H], F32)               # rows j<6: W1[f, j//3]
    w1tp = inp("w1tp", [P, H], F32)               # rows 0:2 = conv1_w^T
    conv2_wT = inp("conv2_wT", [P, H], F32)
    np1_wT_h = inp("np1_wT_h", [P, H], F32)
    np1_wT_z = inp("np1_wT_z", [P, H], F32)
    np2_wT = inp("np2_wT", [P, H], F32)
    src_wT = inp("src_wT", [P, H], F32)
    tgt_wT = inp("tgt_wT", [P, H], F32)
    wut = inp("wut", [P, H], F32)
    wvt = inp("wvt", [P, H], F32)
    w2_col = inp("w2_col", [P, 1], F32)           # inv2_w as a column
    crow_col_i = inp("crow_col_i", [P, 1], F32)   # inv1_wz@z + inv1_b (host)
    z_rep = inp("z_rep", [P, VS], F32)            # z^T broadcast along nodes
    b1_rep = inp("b1_rep", [P, H], F32)
    b2_col = inp("b2_col", [P, 1], F32)
    np1_b_col = inp("np1_b_col", [P, 1], F32)
    np2_b_col = inp("np2_b_col", [P, 1], F32)
    rep16 = inp("rep16", [16, P], F32)            # 16->128 partition replicator

    def outp(name, shape, dt):
        return nc.dram_tensor(name, shape, dt, kind="ExternalOutput")

    o_vals = outp("o_vals", [VS, 2], F32)
    o_logit = outp("o_logit", [VS, 2], F32)
    o_idx = outp("o_idx", [VS, 2], I32)
    o_bit = outp("o_bit", [VS, 2], I32)
    o_valid = outp("o_valid", [VS, 2], I32)
    DBG = bool(int(os.environ.get("KERNEL_DBG", "0")))
    if DBG:
        o_dbg_gidx = outp("o_dbg_gidx", [VS, 16], I32)
        o_dbg_pg = outp("o_dbg_pg", [VS, 2, H], F32)
        o_dbg_h = outp("o_dbg_h", [VS, H], F32)

    cid = nc.partition_id()
    NCH = NT // CHT                                # streamed cmat chunks

    with tile.TileContext(nc) as tc:
        with tc.tile_pool(name="const", bufs=1) as cst, \
             tc.tile_pool(name="work", bufs=2) as wrk, \
             tc.tile_pool(name="stream", bufs=2) as stm, \
             tc.tile_pool(name="cstream", bufs=4) as csm, \
             tc.tile_pool(name="ystream", bufs=2) as ysm, \
             tc.tile_pool(name="big", bufs=1) as big, \
             tc.tile_pool(name="score", bufs=2) as scr, \
             tc.tile_pool(name="ps", bufs=2, space="PSUM") as ps, \
             tc.tile_pool(name="ps_s", bufs=3, space="PSUM") as pss, \
             tc.tile_pool(name="ps_acc", bufs=1, space="PSUM") as psa, \
             tc.tile_pool(name="dram", bufs=1, space="DRAM") as dram:

            # ---------------- phase 0: constants ----------------
            ident = cst.tile([P, P], F32)
            make_identity(nc, ident[:])
            junk_bf = cst.tile([P, TS], BF16)
            nc.gpsimd.memset(junk_bf[:], 0.0)

            def warm_pe(stat_ap, m, n):
                """Back-to-back junk matmuls to trip the HAM clock gate to
                8/8 (2.4GHz) right before a gappy matmul stream starts; a
                stream whose bursts are shorter than the 3.4us activity
                window otherwise runs cold (1.2GHz) forever."""
                wps = ps.tile([P, TS], F32, tag="mm512", name="warm")
                for i in range(n):
                    nc.tensor.matmul(wps[0:m, 0:TS], stat_ap, junk_bf[:],
                                     start=(i == 0), stop=(i == n - 1))

            _eng = [nc.sync, nc.scalar, nc.gpsimd]
            _ldi = [0]

            def load(handle, shape, dt, pool=cst):
                nm = f"ld_{handle.name}"
                t = pool.tile(shape, dt, name=nm, tag=nm)
                e = _eng[_ldi[0] % 3]
                _ldi[0] += 1
                e.dma_start(t[:], handle[tuple(slice(0, s) for s in shape)])
                return t

            # conv1-stream critical path loads first
            pall_sb = load(p_all, [P, NT, 2], F32)
            w1t6_sb = load(w1t6, [P, H], F32)
            w1tp_sb = load(w1tp, [P, H], F32)
            xinv = big.tile([P, VS], F32, name="xinv", tag="sG")
            nc.sync.dma_start(xinv[:], x_ownT_div[0:P, 0:VS])
            # p6 = rsq * x (all nodes, host-premultiplied), 3-way bf16 split
            p6 = big.tile([P, NT, 6], BF16)

            # remaining constant loads
            c2wt_sb = load(conv2_wT, [P, H], F32)
            np1h_sb = load(np1_wT_h, [P, H], F32)
            np1z_sb = load(np1_wT_z, [P, H], F32)
            np2_sb = load(np2_wT, [P, H], F32)
            srcw_sb = load(src_wT, [P, H], F32)
            tgtw_sb = load(tgt_wT, [P, H], F32)
            wut_sb = load(wut, [P, H], F32)
            wvt_sb = load(wvt, [P, H], F32)
            w2col_sb = load(w2_col, [P, 1], F32)
            crow_col = load(crow_col_i, [P, 1], F32)
            b1rep_sb = load(b1_rep, [P, H], F32)
            b2col_sb = load(b2_col, [P, 1], F32)
            np1b_sb = load(np1_b_col, [P, 1], F32)
            np2b_sb = load(np2_b_col, [P, 1], F32)
            rep16_sb = load(rep16, [16, P], F32)
            rsqot_sb = load(rsq_ot, [P, JB], F32)
            dotb_sb = load(dotb, [P, JB], F32)
            dot_sb = load(dot_t, [P, JB], F32)
            tot_sb = load(typ_t, [P, JB], F32)
            degrow = cst.tile([1, VS], F32)
            nc.sync.dma_start(degrow[:], rsq_own[0:1, 0:VS])
            rsq_rep = cst.tile([P, VS], F32)
            inv_rep = cst.tile([P, VS], F32)
            nc.gpsimd.partition_broadcast(rsq_rep[:], degrow[:])
            nc.scalar.dma_start(degrow[:], inv_own[0:1, 0:VS])
            nc.gpsimd.partition_broadcast(inv_rep[:], degrow[:])

            # internal DRAM
            ag1_in = dram.tile([P, JB * NSPL * H], BF16)
            ag1_out = dram.tile([NCORES, P * JB * NSPL * H], BF16,
                                addr_space="Shared")
            ag2_in = dram.tile([1, 2 * VS * H], F32)
            ag2_out = dram.tile([NCORES, 2 * VS * H], F32, addr_space="Shared")
            idx_dram = dram.tile([VS, 2], I16)

            # ---------------- phase 1: conv1 ----------------
            # q stream: psum[j, v] accumulates sum_u p6[u, j] * C[v, u]
            q_ps = [psa.tile([6, TS], F32, tag=f"qaps{h}", name=f"qaps{h}")
                    for h in range(2)]
            for k2 in range(NCH):
                cch = csm.tile([P, CHT, VS], BF16, tag="cchunk")
                _eng[k2 % 3].dma_start(
                    cch[:], cmat_t[:, k2 * CHT:(k2 + 1) * CHT, :])
                if k2 == 0:
                    # p6 split sits here so its DVE work overlaps chunk-0 DMA
                    sp_a = wrk.tile([P, NT], F32, tag="sp_a")
                    sp_b = wrk.tile([P, NT], F32, tag="sp_b")
                    for fc in range(2):
                        _split3(nc,
                                (p6[:, :, 3 * fc], p6[:, :, 3 * fc + 1],
                                 p6[:, :, 3 * fc + 2]),
                                pall_sb[:, :, fc], sp_a[:], sp_b[:])
                    # warm the PE right as the stream begins (dep on p6)
                    warm_pe(p6[:, 0, :], 6, 12)
                for t in range(CHT):
                    k = CHT * k2 + t
                    for h in range(2):
                        nc.tensor.matmul(q_ps[h][:], p6[:, k, :],
                                         cch[:, t, h * TS:(h + 1) * TS],
                                         start=(k == 0), stop=(k == NT - 1))
            qcomp = big.tile([P, VS], F32, tag="sA")
            for h in range(2):
                nc.scalar.copy(qcomp[0:6, h * TS:(h + 1) * TS], q_ps[h][:])
            nc.vector.tensor_tensor(qcomp[0:6, :], qcomp[0:6, :],
                                    rsq_rep[0:6, :], OP.mult)

            h1_own = big.tile([P, JB, H], F32, tag="sD")
            for v8 in range(JB):
                hps = ps.tile([P, TS], F32, tag="mm512")
                nc.tensor.matmul(hps[:, 0:H], qcomp[0:6, v8 * P:(v8 + 1) * P],
                                 w1t6_sb[0:6, :], start=True, stop=False)
                nc.tensor.matmul(hps[:, 0:H], xinv[0:2, v8 * P:(v8 + 1) * P],
                                 w1tp_sb[0:2, :], start=False, stop=True)
                nc.vector.tensor_tensor(h1_own[:, v8, :], hps[:, 0:H],
                                        b1rep_sb[:], OP.add)
                nc.vector.tensor_scalar(h1_own[:, v8, :], h1_own[:, v8, :],
                                        0.0, None, op0=OP.max)

            # h1_own^T (feature-major) for conv2 self-loop term
            h1T = big.tile([P, VS], F32, tag="sB")
            for v8 in range(JB):
                tps = ps.tile([P, TS], F32, tag="mm512")
                nc.tensor.transpose(tps[:, 0:P], h1_own[:, v8, :], ident[:])
                nc.scalar.copy(h1T[:, v8 * P:(v8 + 1) * P], tps[:, 0:P])

            # xw2 = h1 @ W2^T: feature-major via xw2T = W2^T-matmul(h1T)
            xw2T = big.tile([P, VS], F32, tag="sC")
            for hh in range(2):
                sl = slice(hh * TS, (hh + 1) * TS)
                xps = ps.tile([P, TS], F32, tag="mm512")
                nc.tensor.matmul(xps[:], c2wt_sb[:], h1T[:, sl],
                                 start=True, stop=True)
                nc.scalar.copy(xw2T[:, sl], xps[:])
            xw2_own = big.tile([P, JB, H], F32, tag="sF")
            for v8 in range(JB):
                tps = ps.tile([P, TS], F32, tag="mm512")
                nc.tensor.transpose(tps[:, 0:P], xw2T[:, v8 * P:(v8 + 1) * P],
                                    ident[:])
                nc.scalar.copy(xw2_own[:, v8, :], tps[:, 0:P])

            # y2 = rsq_own * xw2, 2-way bf16 split -> AG1 (written per half so
            # the collective's input wait starts draining early)
            y2s = big.tile([P, JB, NSPL, H], BF16, tag="sE")
            sy_a = wrk.tile([P, H], F32, tag="sy_a")
            y2t = wrk.tile([P, H], F32, tag="y2t")
            for t in range(JB):
                rc = wrk.tile([P, 1], F32, tag="rsqcol")
                nc.vector.tensor_copy(rc[:], rsqot_sb[:, t:t + 1])
                nc.vector.tensor_scalar(y2t[:], xw2_own[:, t, :], rc[:], None,
                                        op0=OP.mult)
                _split2(nc, (y2s[:, t, 0, :], y2s[:, t, 1, :]), y2t[:],
                        sy_a[:])
                if t == 3:
                    nc.gpsimd.dma_start(
                        ag1_in[:, 0:JB * NSPL * H // 2],
                        y2s[:, 0:4, :, :].rearrange("p t s f -> p (t s f)"))
            nc.gpsimd.dma_start(
                ag1_in[:, JB * NSPL * H // 2:],
                y2s[:, 4:8, :, :].rearrange("p t s f -> p (t s f)"))
            nc.gpsimd.collective_compute(
                "AllGather", OP.bypass,
                replica_groups=[list(range(NCORES))],
                ins=[ag1_in[:].opt()], outs=[ag1_out[:].opt()])

            zrep_sb = big.tile([P, VS], F32, tag="sZ")
            nc.scalar.dma_start(zrep_sb[:], z_rep[:, :])

            # ---------------- phase 2: conv2 ----------------
            a2_ps = [psa.tile([P, TS], F32, tag=f"qaps{h}", name=f"a2ps{h}")
                     for h in range(2)]
            for k2 in range(NCH):
                cch = csm.tile([P, CHT, VS], BF16, tag="cchunk")
                _eng[k2 % 3].dma_start(
                    cch[:], cmat_t[:, k2 * CHT:(k2 + 1) * CHT, :])
                if (k2 * CHT) % JB == 0:
                    c8 = (k2 * CHT) // JB
                    ych = ysm.tile([P, JB, NSPL, H], BF16, tag="ychunk")
                    nc.gpsimd.dma_start(
                        ych[:], ag1_out[c8:c8 + 1, :].rearrange(
                            "q (p t s f) -> p (q t) s f", p=P, s=NSPL, f=H))
                    if k2 == 0:
                        # re-warm the PE coming out of the AG1 idle window
                        warm_pe(ych[:, 0, 0, :], P, 12)
                for t4 in range(CHT):
                    k = CHT * k2 + t4
                    tl = k % JB                    # tile within shard
                    for s in range(NSPL):
                        for h in range(2):
                            nc.tensor.matmul(a2_ps[h][:], ych[:, tl, s, :],
                                             cch[:, t4, h * TS:(h + 1) * TS],
                                             start=(k == 0 and s == 0),
                                             stop=(k == NT - 1 and
                                                   s == NSPL - 1))
            h2T = big.tile([P, VS], F32, tag="sA")
            for h in range(2):
                sl = slice(h * TS, (h + 1) * TS)
                t1 = wrk.tile([P, TS], F32, tag="c2a", bufs=1)
                nc.vector.tensor_tensor(t1[:], a2_ps[h][:], rsq_rep[:, sl],
                                        OP.mult)
                t2 = wrk.tile([P, TS], F32, tag="c2b", bufs=1)
                nc.vector.tensor_tensor(t2[:], xw2T[:, sl], inv_rep[:, sl],
                                        OP.mult)
                nc.vector.tensor_tensor(t1[:], t1[:], t2[:], OP.add)
                nc.scalar.activation(h2T[:, sl], t1[:], AF.Relu,
                                     bias=b2col_sb[:], scale=1.0)

            # ---------------- phase 3: node_proj + AG2 pack ----------------
            # AG2 payload: [0,VS*H) = h node-major [p][t][f],
            # [VS*H, 2*VS*H) = Hs^T (src_w @ h^T) feature-major [f][v].
            # Produced and written to DRAM per 512-column half so the
            # collective's input wait is near-zero when it triggers.
            a1T = big.tile([P, VS], F32, tag="sB")
            hT = big.tile([P, VS], F32, tag="sH")
            h_own = big.tile([P, JB, H], F32, tag="sD")
            hsow = big.tile([P, VS], F32, tag="sC")
            ag2_hview = ag2_in[0:1, 0:VS * H].rearrange(
                "q (p t f) -> p (q t) f", p=P, f=H)
            ag2_sview = ag2_in[0:1, VS * H:2 * VS * H].rearrange(
                "q (f v) -> f q v", f=P)
            for h in range(2):
                sl = slice(h * TS, (h + 1) * TS)
                nps = ps.tile([P, TS], F32, tag="mm512")
                nc.tensor.matmul(nps[:], np1h_sb[:], h2T[:, sl],
                                 start=True, stop=False)
                nc.tensor.matmul(nps[:], np1z_sb[:], zrep_sb[:, sl],
                                 start=False, stop=True)
                nc.scalar.activation(a1T[:, sl], nps[:], AF.Relu,
                                     bias=np1b_sb[:], scale=1.0)
                nps2 = ps.tile([P, TS], F32, tag="mm512")
                nc.tensor.matmul(nps2[:], np2_sb[:], a1T[:, sl],
                                 start=True, stop=True)
                nc.scalar.activation(hT[:, sl], nps2[:], AF.Identity,
                                     bias=np2b_sb[:], scale=1.0)
                sps = ps.tile([P, TS], F32, tag="mm512")
                nc.tensor.matmul(sps[:], srcw_sb[:], hT[:, sl],
                                 start=True, stop=True)
                nc.scalar.copy(hsow[:, sl], sps[:])
                nc.sync.dma_start(ag2_sview[:, 0, sl], hsow[:, sl])
                for v8 in range(4 * h, 4 * h + 4):
                    tps = ps.tile([P, TS], F32, tag="mm512")
                    nc.tensor.transpose(tps[:, 0:P],
                                        hT[:, v8 * P:(v8 + 1) * P], ident[:])
                    nc.scalar.copy(h_own[:, v8, :], tps[:, 0:P])
                nc.gpsimd.dma_start(ag2_hview[:, 4 * h:4 * h + 4, :],
                                    h_own[:, 4 * h:4 * h + 4, :])
            if DBG:
                nc.sync.dma_start(
                    o_dbg_h.ap().rearrange("(t p) f -> p t f", p=P),
                    h_own[:])
            nc.gpsimd.collective_compute(
                "AllGather", OP.bypass,
                replica_groups=[list(range(NCORES))],
                ins=[ag2_in[:].opt()], outs=[ag2_out[:].opt()])

            # ---------------- phase 4: gathered tensors ----------------
            # own scorer-row chunks first (tiny), then Hs^T section loads
            htT = big.tile([P, JB, P], F32, tag="sF")
            qcT = big.tile([P, JB, P], F32, tag="sG")
            hvs = {}
            for i, j in enumerate(TORD):
                hv = stm.tile([P, 1, P], F32, tag="hv", bufs=8)
                _eng[i % 3].dma_start(
                    hv[:], ag2_out[j:j + 1, 0:VS * H].rearrange(
                        "q (p t f) -> p (q t) f", p=P, f=H)[:, ds(cid, 1), :])
                hvs[j] = hv
            hsT = big.tile([P, N], F32, tag="big_hsT")
            for c8 in range(NCORES):
                _eng[c8 % 3].dma_start(
                    hsT[:, c8 * VS:(c8 + 1) * VS],
                    ag2_out[c8:c8 + 1, VS * H:2 * VS * H].rearrange(
                        "q (f v) -> f q v", f=P)[:, 0, :])
            for j in TORD:
                tps = ps.tile([P, TS], F32, tag="mm512")
                nc.tensor.transpose(tps[:, 0:P], hvs[j][:, 0, :], ident[:])
                hvT = stm.tile([P, P], F32, tag="hvT", bufs=1)
                nc.scalar.copy(hvT[:], tps[:, 0:P])
                hps = ps.tile([P, TS], F32, tag="mm512")
                nc.tensor.matmul(hps[:, 0:P], tgtw_sb[:], hvT[:],
                                 start=True, stop=True)
                nc.scalar.copy(htT[:, j, :], hps[:, 0:P])
                qps = ps.tile([P, TS], F32, tag="mm512")
                nc.tensor.matmul(qps[:, 0:P], wvt_sb[:], hvT[:],
                                 start=True, stop=True)
                nc.scalar.copy(qcT[:, j, :], qps[:, 0:P])

            # ---------------- valid flags (early, independent) -------------
            v0 = wrk.tile([P, JB], F32, tag="v0")
            v1 = wrk.tile([P, JB], F32, tag="v1")
            tns = wrk.tile([P, JB], F32, tag="tns")
            nc.vector.tensor_scalar(v0[:], dot_sb[:], 1.0, None, op0=OP.is_ge)
            nc.vector.tensor_scalar(tns[:], tot_sb[:], 0.0, None,
                                    op0=OP.not_equal)
            nc.vector.tensor_tensor(v0[:], v0[:], tns[:], OP.mult)
            nc.vector.tensor_scalar(tns[:], tot_sb[:], 2.0, None,
                                    op0=OP.is_equal)
            nc.vector.tensor_tensor(v1[:], v0[:], tns[:], OP.mult)
            valid_i32 = big.tile([P, JB, 2], I32)
            nc.vector.tensor_copy(valid_i32[:, :, 0], v0[:])
            nc.vector.tensor_copy(valid_i32[:, :, 1], v1[:])
            nc.scalar.dma_start(
                o_valid.ap().rearrange("(j p) s -> p j s", p=P), valid_i32[:])

            # ------- phase 5: scores + top-2, folded gather + inv MLP -------
            # software-pipelined 3 deep: scores(j) || idx-roundtrip+gather
            # (j-1) || inversion-MLP(j-2), so no engine queue head-of-line
            # blocks on a cross-engine latency chain.
            hrows = ag2_out[:].rearrange("c (r f) -> (c r) f", f=H)
            pgs = [None] * JB
            depblks = {}

            def fetch_dep(j, eng):
                span = T[j] - TM[j]
                if span <= 0:
                    depblks[j] = None
                    return
                dpb = wrk.tile([P, MS, TS], BF16, tag="depblk")
                eng.dma_start(
                    dpb[:, 0:span, :],
                    depneg_rep[:, TM[j] * TS:T[j] * TS].rearrange(
                        "p (m s) -> p m s", s=TS))
                depblks[j] = dpb

            def roundtrip_b(j):
                # idx readback in gather layout:
                # g16[q, s*8+e] = idx[e*16+q, s]
                g16 = wrk.tile([16, 2, 8], I16, tag="g16")
                nc.scalar.dma_start(
                    g16[:], idx_dram[j * P:(j + 1) * P, :].rearrange(
                        "(e q) s -> q s e", e=8, q=16))
                g16f = wrk.tile([16, 16], F32, tag="g16f")
                nc.gpsimd.tensor_copy(g16f[:],
                                      g16[:].rearrange("q s e -> q (s e)"))
                grep_ps = ps.tile([P, TS], F32, tag="mm512")
                nc.tensor.matmul(grep_ps[:, 0:16], rep16_sb[:], g16f[:],
                                 start=True, stop=True)
                gidx = stm.tile([P, 16], I16, tag="gidx")
                nc.vector.tensor_copy(gidx[:], grep_ps[:, 0:16])
                pg = stm.tile([P, 2, H], F32, tag="pg", bufs=3)
                nc.gpsimd.dma_gather(
                    out_ap=pg[:, 0:2, :],
                    in_ap=hrows[0:NCORES * 2 * VS, 0:H],
                    idxs_ap=gidx[:, 0:16],
                    num_idxs=256, num_idxs_reg=256, elem_size=H)
                pgs[j] = pg
                if DBG:
                    gx32 = wrk.tile([P, 16], I32, tag="gx32")
                    nc.vector.tensor_copy(gx32[:], gidx[:])
                    nc.sync.dma_start(
                        o_dbg_gidx[j * P:(j + 1) * P, :], gx32[:])
                    nc.scalar.dma_start(
                        o_dbg_pg[j * P:(j + 1) * P, :, :], pg[:, 0:2, :])

            def mlp_block(j):
                pg = pgs[j]
                hgT = wrk.tile([P, 2, P], F32, tag="hgT")
                for s in range(2):
                    tps = ps.tile([P, TS], F32, tag="mm512")
                    nc.tensor.transpose(tps[:, 0:P], pg[:, s, :], ident[:])
                    nc.scalar.copy(hgT[:, s, :], tps[:, 0:P])
                pu_ps = ps.tile([P, TS], F32, tag="mm512")
                nc.tensor.matmul(pu_ps[:, 0:2 * P], wut_sb[:],
                                 hgT[:].rearrange("p a b -> p (a b)"),
                                 start=True, stop=True)
                pre = wrk.tile([P, 2, P], F32, tag="pre2")
                nc.vector.tensor_tensor(
                    pre[:],
                    pu_ps[:, 0:2 * P].rearrange("p (a b) -> p a b", a=2),
                    qcT[:, j:j + 1, :].to_broadcast([P, 2, P]),
                    OP.add)
                nc.scalar.activation(
                    pre[:].rearrange("p a b -> p (a b)"),
                    pre[:].rearrange("p a b -> p (a b)"),
                    AF.Relu, bias=crow_col[:], scale=1.0)
                lg_ps = ps.tile([P, TS], F32, tag="mm512")
                nc.tensor.matmul(lg_ps[0:1, 0:2 * P], w2col_sb[:],
                                 pre[:].rearrange("p a b -> p (a b)"),
                                 start=True, stop=True)
                lg_sb = wrk.tile([1, 2 * P], F32, tag="lgsb")
                nc.vector.tensor_scalar(lg_sb[:], lg_ps[0:1, 0:2 * P],
                                        float(inv2_b_val), None, op0=OP.add)
                bt_sb = wrk.tile([1, 2 * P], I32, tag="btsb")
                nc.vector.tensor_scalar(bt_sb[:], lg_sb[:], 0.0, None,
                                        op0=OP.is_gt)
                nc.sync.dma_start(
                    o_logit[j * P:(j + 1) * P, :].rearrange("p s -> s p"),
                    lg_sb[0:1, :].rearrange("z (s p) -> z s p", s=2))
                nc.scalar.dma_start(
                    o_bit[j * P:(j + 1) * P, :].rearrange("p s -> s p"),
                    bt_sb[0:1, :].rearrange("z (s p) -> z s p", s=2))

            fetch_dep(TORD[0], nc.sync)
            for bi, j in enumerate(TORD):
                if bi + 1 < JB:
                    fetch_dep(TORD[bi + 1], _eng[bi % 3])
                sbuf_row = scr.tile([P, N], F32, tag="srow")
                collect = wrk.tile([P, 16 * 8], F32, tag="collect")
                dpb = depblks[j]
                for t in range(T[j]):
                    sl = slice(t * TS, (t + 1) * TS)
                    sps = pss.tile([P, TS], F32, tag="scoreps")
                    nc.tensor.matmul(sps[:], htT[:, j, :], hsT[:, sl],
                                     start=True, stop=True)
                    if t < TM[j]:
                        nc.scalar.copy(sbuf_row[:, sl], sps[:])
                    else:
                        # masked = min(S, (thr_v - depth_u + 0.5)*1e30)
                        nc.vector.scalar_tensor_tensor(
                            sbuf_row[:, sl], dpb[:, t - TM[j], :],
                            dotb_sb[:, j:j + 1], sps[:],
                            op0=OP.add, op1=OP.min)
                    nc.vector.max(out=collect[:, 8 * t:8 * t + 8],
                                  in_=sbuf_row[:, sl])
                if bi >= 1:
                    roundtrip_b(TORD[bi - 1])
                if bi >= 2:
                    mlp_block(TORD[bi - 2])
                mx = wrk.tile([P, 8], F32, tag="mx")
                mi = wrk.tile([P, 8], U32, tag="mi")
                nc.vector.max(out=mx[:], in_=collect[:, 0:8 * T[j]])
                nc.vector.max_index(out=mi[:], in_max=mx[:],
                                    in_values=sbuf_row[:, 0:T[j] * TS])
                idx32 = wrk.tile([P, 2], I32, tag="idx32")
                nc.vector.tensor_copy(idx32[:], mi[:, 0:2])
                nc.scalar.dma_start(o_vals[j * P:(j + 1) * P, :], mx[:, 0:2])
                nc.sync.dma_start(o_idx[j * P:(j + 1) * P, :], idx32[:])
                # gather-row remap: node u -> c*2048 + (u&127)*8 + ((u>>7)&7)
                # = 8u + 2040c - 1023a with a=u>>7, c=u>>10 (avoids the
                # microcoded bitwise_and slow path, ~4us per op on DVE)
                ra = wrk.tile([P, 2], U32, tag="ra")
                nc.vector.tensor_scalar(ra[:], mi[:, 0:2], 7, None,
                                        op0=OP.logical_shift_right)
                rc2 = wrk.tile([P, 2], U32, tag="rc2")
                nc.vector.tensor_scalar(rc2[:], mi[:, 0:2], 10, None,
                                        op0=OP.logical_shift_right)
                rb = wrk.tile([P, 2], U32, tag="rb")
                nc.vector.tensor_scalar(rb[:], mi[:, 0:2], 3, None,
                                        op0=OP.logical_shift_left)
                nc.vector.scalar_tensor_tensor(rb[:], rc2[:], 2040, rb[:],
                                               op0=OP.mult, op1=OP.add)
                nc.vector.tensor_scalar(ra[:], ra[:], 1023, None,
                                        op0=OP.mult)
                nc.vector.tensor_tensor(rb[:], rb[:], ra[:], OP.subtract)
                i16t = wrk.tile([P, 2], I16, tag="i16t")
                nc.vector.tensor_copy(i16t[:], rb[:])
                nc.sync.dma_start(idx_dram[j * P:(j + 1) * P, :], i16t[:])
            roundtrip_b(TORD[-1])
            mlp_block(TORD[-2])
            mlp_block(TORD[-1])

    nc.compile()
    return nc


# --------------------------------------------------------------------------
# host wrapper
# --------------------------------------------------------------------------

def _tiled(v):
    """[N] -> [128, N//128] with v_t[p, t] = v[t*128+p]."""
    return np.ascontiguousarray(v.reshape(-1, P).T)


def kernel(**inputs):
    global LAST_RESULT
    x = np.asarray(inputs["x"], np.float32)
    z = np.asarray(inputs["z"], np.float32)
    ei = np.asarray(inputs["edge_index"]).astype(np.int64)
    depth = np.asarray(inputs["node_depth"]).astype(np.int64)
    w = {k: np.asarray(v, np.float32) for k, v in inputs.items()
         if k.endswith("_w") or k.endswith("_b")}

    src, dst = ei[0], ei[1]

    # graph structure prep (host): edge-count matrix C^T[u, v] and degrees
    C_t = np.zeros((N, N), dtype=np.float32)
    np.add.at(C_t, (src, dst), 1.0)
    deg = (np.bincount(dst, minlength=N) + 1).astype(np.float32)
    rsq = (np.float32(1.0) / np.sqrt(deg, dtype=np.float32)).astype(np.float32)
    inv = (np.float32(1.0) / deg).astype(np.float32)

    dep_f = depth.astype(np.float32)
    # prefix cutoffs (depth is sorted): candidate u valid iff
    # depth[u] <= depth[v]+1
    cut = np.searchsorted(depth, depth + DEPTH_PERTURB - 1, side="right")
    T, TM = [], []
    for j in range(JB):
        c_hi = int(cut[(j + 1) * (P * NCORES) - 1])
        c_lo = int(cut[j * (P * NCORES)])
        T.append(max(1, -(-c_hi // TS)))
        TM.append(min(c_lo // TS, T[-1]))
    TORD = sorted(range(JB), key=lambda j: -T[j])
    MS = max(1, max(T[j] - TM[j] for j in range(JB)))

    inv2_b_val = float(w["inv2_b"][0])
    nc = build_program(T, TM, TORD, MS, inv2_b_val)

    # replicated host-side tensors
    crow = (w["inv1_w"][:, 2 * H:].astype(np.float64) @ z.astype(np.float64)
            + w["inv1_b"].astype(np.float64)).astype(np.float32)
    p_host = rsq[:, None] * x                                # [N, 2] f32
    rep = {
        "p_all": np.ascontiguousarray(
            p_host.reshape(NT, P, 2).transpose(1, 0, 2)),
        "depneg_rep": np.ascontiguousarray(np.broadcast_to(
            ((0.5 - dep_f) * BIGM)[None, :], (P, N))).astype(
                ml_dtypes.bfloat16),
        "w1t6": _pad128(np.stack([w["conv1_w"].T[fc]
                                  for fc in (0, 0, 0, 1, 1, 1)], 0)),
        "w1tp": _pad128(w["conv1_w"].T),
        "conv2_wT": np.ascontiguousarray(w["conv2_w"].T),
        "np1_wT_h": np.ascontiguousarray(w["np1_w"][:, :H].T),
        "np1_wT_z": np.ascontiguousarray(w["np1_w"][:, H:].T),
        "np2_wT": np.ascontiguousarray(w["np2_w"].T),
        "src_wT": np.ascontiguousarray(w["src_w"].T),
        "tgt_wT": np.ascontiguousarray(w["tgt_w"].T),
        "wut": np.ascontiguousarray(w["inv1_w"][:, :H].T),
        "wvt": np.ascontiguousarray(w["inv1_w"][:, H:2 * H].T),
        "w2_col": np.ascontiguousarray(w["inv2_w"][0][:, None]),
        "crow_col_i": np.ascontiguousarray(crow[:, None]),
        "z_rep": np.ascontiguousarray(np.broadcast_to(z[:, None], (P, VS))),
        "b1_rep": np.ascontiguousarray(
            np.broadcast_to(w["conv1_b"][None, :], (P, H))),
        "b2_col": np.ascontiguousarray(w["conv2_b"][:, None]),
        "np1_b_col": np.ascontiguousarray(w["np1_b"][:, None]),
        "np2_b_col": np.ascontiguousarray(w["np2_b"][:, None]),
        "rep16": np.ascontiguousarray(
            (np.arange(P)[None, :] % 16 == np.arange(16)[:, None])
            .astype(np.float32)),
    }

    in_maps = []
    for c in range(NCORES):
        sh = slice(c * VS, (c + 1) * VS)
        own_rows = np.concatenate(
            [np.arange(P * (JB * j + c), P * (JB * j + c) + P)
             for j in range(JB)])
        m = dict(rep)
        m["cmat_t"] = np.ascontiguousarray(
            C_t[:, sh].reshape(NT, P, VS).transpose(1, 0, 2)).astype(
                ml_dtypes.bfloat16)
        m["x_ownT_div"] = _pad128((x[sh] / deg[sh][:, None]).T)
        m["rsq_own"] = np.ascontiguousarray(rsq[sh][None, :])
        m["inv_own"] = np.ascontiguousarray(inv[sh][None, :])
        m["rsq_ot"] = _tiled(rsq[sh])
        m["dotb"] = _tiled((dep_f[own_rows] + 1.0) * np.float32(BIGM))
        m["dot_t"] = _tiled(dep_f[own_rows])
        m["typ_t"] = _tiled(x[own_rows, 0])
        in_maps.append(m)

    trace = bool(int(os.environ.get("KERNEL_PROFILE", "0")))
    res = bass_utils.run_bass_kernel_spmd(nc, in_maps,
                                          core_ids=list(range(NCORES)),
                                          trace=trace)
    LAST_RESULT = res

    top_vals = np.zeros((N, 2), np.float32)
    inv_logit = np.zeros((N, 2), np.float32)
    top_idx = np.zeros((N, 2), np.int32)
    inv_bit = np.zeros((N, 2), np.int32)
    valid = np.zeros((N, 2), bool)
    for c in range(NCORES):
        r = res.results[c]
        for j in range(JB):
            g = JB * j + c
            rows = slice(P * g, P * g + P)
            lrows = slice(P * j, P * j + P)
            top_vals[rows] = r["o_vals"][lrows]
            inv_logit[rows] = r["o_logit"][lrows]
            top_idx[rows] = r["o_idx"][lrows]
            inv_bit[rows] = r["o_bit"][lrows]
            valid[rows] = r["o_valid"][lrows].astype(bool)
    return top_vals, inv_logit, top_idx, inv_bit, valid


def _pad128(arr2):
    """Pad leading dim of a [k, M] array to [128, M] with zeros."""
    out = np.zeros((P, arr2.shape[1]), np.float32)
    out[:arr2.shape[0]] = arr2
    return out
